# revision 1
# baseline (speedup 1.0000x reference)
"""DDFConvGuidedFilter Trainium2 kernel.

Data-parallel over batch: 16 images -> 8 cores x 2 images.

Per image (512x512, C=3), three guided-filter stages (k=3,7,15; eps=.16,.04,.01):
  s = sum_c x_c ; q = sum_c x_c^2
  per stage:  mean = box(s)/Nc ; Q = box(q)/Nc        (Nc = 3*cnt_h*cnt_w)
              var = Q - mean^2 ; r = 1/(var+eps)
              A3 = 3 - 3*eps*r (= 3A);  rm3 = 3*eps*r*mean (= 3b)
              mA = box(A3) ; mB = box(rm3)            (= C*box(A), C*box(b))
              next: s' = mA*s + 3mB ; q' = mA^2 q + 2 mA mB s + 3 mB^2
  F_j = G_j*x + H_j with G1=mA1, H1=mB1, G2=mA2*G1, H2=mA2*H1+mB2, ...
  out_o = sum_jc w1[o,3(j-1)+c] * (u_j * x_c)  + beta_o2*v2 + beta_o3*v3 - beta_o1*H1
    u1=1-G1, u2=G1-G2, u3=(1-mA3)*G2, v2=H1-H2, v3=(1-mA3)*H2-mB3

Box filters: horizontal pass = DVE tensor_tensor_scan (running sum of
x[t+pad]-x[t-pad-1], started `pad` early); vertical pass = PE matmul against
banded 128x128 Toeplitz blocks (with 1/Nc row-normalization folded into the
s/q bands; w-edge correction applied to 7 edge columns afterwards).
"""

import numpy as np

H = W = 512
C = 3
NB = 4  # h blocks of 128
OFF = 16  # left zero margin in padded tiles
BW = OFF + W + 8  # padded tile width = 536
STAGES = [(3, 0.16), (7, 0.04), (15, 0.01)]
N_CORES = 8
IMGS_PER_CORE = 2


def _cnt(k, n=512):
    i = np.arange(n)
    p = (k - 1) // 2
    return np.minimum(i + p + 1, np.minimum(2 * p + 1, n - i + p)).astype(np.float64)


def _band_block(k, dj, row_scale=None):
    """lhsT[kk, m] = scale(h_out) * 1(|128*dj + kk - m| <= pad)."""
    pad = (k - 1) // 2
    m = np.arange(128)
    kk = np.arange(128)[:, None]
    mat = (np.abs(128 * dj + kk - m) <= pad).astype(np.float64)
    if row_scale is not None:
        mat = mat * row_scale[None, :]
    return mat


def _make_consts():
    """Band matrices [24,128,128] and edge-gamma tiles [3,2,128,4,7]."""
    bands = []
    gedges = np.zeros((3, 2, 128, NB, 7), np.float64)
    for si, (k, eps) in enumerate(STAGES):
        ch = _cnt(k)
        alpha = 1.0 / (3.0 * ch * k)  # row scale: 1/(3*cnt_h(h)*k)
        # sq bands: diag0, diagM, diag3, upM, dnM
        bands.append(_band_block(k, 0, alpha[0:128]))
        bands.append(_band_block(k, 0, alpha[128:256]))
        bands.append(_band_block(k, 0, alpha[384:512]))
        bands.append(_band_block(k, 1, alpha[128:256]))  # interior rows
        bands.append(_band_block(k, -1, alpha[128:256]))
        # A/rm bands: unnormalized
        bands.append(_band_block(k, 0))
        bands.append(_band_block(k, 1))
        bands.append(_band_block(k, -1))
        cw = _cnt(k)
        gl = k / cw[0:7]
        gr = k / cw[505:512]
        gedges[si, 0] = np.broadcast_to(gl[None, None, :], (128, NB, 7))
        gedges[si, 1] = np.broadcast_to(gr[None, None, :], (128, NB, 7))
    return np.stack(bands).astype(np.float32), gedges.astype(np.float32)


# band index helpers: per stage si, base = si*8
SQ_DIAG = {0: 0, 1: 1, 2: 1, 3: 2}  # per out-block
SQ_UP, SQ_DN = 3, 4
A_DIAG, A_UP, A_DN = 5, 6, 7


def _make_diags(w1):
    """Conv lhsT diag matrices [36,128,128]: 27 p-term + 9 v-term."""
    eye = np.eye(128, dtype=np.float64)
    mats = []
    for o in range(3):
        for j in range(3):
            for c in range(3):
                mats.append(w1[o, 3 * j + c] * eye)
    beta = w1.reshape(3, 3, 3).sum(axis=2)  # [o, j]
    for o in range(3):
        mats.append(beta[o, 1] * eye)   # v2
        mats.append(beta[o, 2] * eye)   # v3
        mats.append(-beta[o, 0] * eye)  # H1
    return np.stack(mats).astype(np.float32)


def _build_program(reps=1):
    import concourse.bacc as bacc
    import concourse.tile as tile
    from concourse import mybir

    F32 = mybir.dt.float32
    ADD = mybir.AluOpType.add
    SUB = mybir.AluOpType.subtract
    MUL = mybir.AluOpType.mult
    SQUARE = mybir.ActivationFunctionType.Square
    COPY = mybir.ActivationFunctionType.Copy

    nc = bacc.Bacc("TRN2", target_bir_lowering=False, debug=False,
                   num_devices=N_CORES)
    xs_ap = nc.dram_tensor("xs", [IMGS_PER_CORE, C, H, W], F32,
                           kind="ExternalInput").ap()
    bands_ap = nc.dram_tensor("bands", [24, 128, 128], F32,
                              kind="ExternalInput").ap()
    gedge_ap = nc.dram_tensor("gedge", [3, 2, 128, NB, 7], F32,
                              kind="ExternalInput").ap()
    diags_ap = nc.dram_tensor("diags", [36, 128, 128], F32,
                              kind="ExternalInput").ap()
    out_ap = nc.dram_tensor("out", [IMGS_PER_CORE, C, H, W], F32,
                            kind="ExternalOutput").ap()

    with tile.TileContext(nc) as tc:
        with (
            tc.tile_pool(name="const", bufs=1) as constp,
            tc.tile_pool(name="fixed", bufs=1) as fixed,
            tc.tile_pool(name="scan", bufs=2) as scanp,
            tc.tile_pool(name="scr", bufs=4) as scr,
            tc.tile_pool(name="pers", bufs=1) as pers,
            tc.tile_pool(name="pp", bufs=4) as pp,
            tc.tile_pool(name="obp", bufs=3) as obp,
            tc.tile_pool(name="psum", bufs=2, space="PSUM") as psum,
        ):
            # ---- constants ----
            bands = constp.tile([128, 24, 128], F32)
            nc.sync.dma_start(bands[:], bands_ap.rearrange("n k m -> k n m"))
            gedge = constp.tile([128, 3, 2, NB, 7], F32)
            nc.sync.dma_start(gedge[:], gedge_ap.rearrange("s e p b j -> p s e b j"))
            diags = constp.tile([128, 36, 128], F32)
            nc.sync.dma_start(diags[:], diags_ap.rearrange("n k m -> k n m"))

            # ---- fixed padded map tiles (shared across stages & images) ----
            s_t = fixed.tile([128, NB, BW], F32, tag="s_t")
            q_t = fixed.tile([128, NB, BW], F32, tag="q_t")
            a_t = fixed.tile([128, NB, BW], F32, tag="a_t")
            rm_t = fixed.tile([128, NB, BW], F32, tag="rm_t")
            for t in (s_t, q_t, a_t, rm_t):
                nc.vector.memset(t[:], 0.0)

            def ctr(t):  # central (data) region of a padded tile
                return t[:, :, OFF:OFF + W]

            def hscan(padded, k):
                """H box pass -> scan tile [128, NB, 520]; data at [:, b, pad:pad+512]."""
                pad = (k - 1) // 2
                n = W + pad
                hs = scanp.tile([128, NB, 520], F32, tag="hs")
                for b in range(NB):
                    nc.vector.tensor_tensor_scan(
                        hs[:, b, 0:n],
                        padded[:, b, OFF:OFF + n],
                        padded[:, b, OFF - k:OFF - k + n],
                        0.0, ADD, SUB,
                    )
                return hs, pad

            def vband(hs, pad, si, sq, tag):
                """V box pass on PE -> psum tile [128, NB, 512]."""
                ps = psum.tile([128, NB, W], F32, tag="ps")
                base = si * 8
                for b in range(NB):
                    js = [j for j in (b - 1, b, b + 1) if 0 <= j < NB]
                    for idx, j in enumerate(js):
                        if sq:
                            bi = base + (SQ_DIAG[b] if j == b else
                                         (SQ_UP if j == b + 1 else SQ_DN))
                        else:
                            bi = base + (A_DIAG if j == b else
                                         (A_UP if j == b + 1 else A_DN))
                        nc.tensor.matmul(
                            ps[:, b, :], bands[:, bi, :],
                            hs[:, j, pad:pad + W],
                            start=(idx == 0), stop=(idx == len(js) - 1),
                        )
                return ps

            def edgefix(ps, si):
                """multiply 7 left/right edge columns by gamma (in-place, PSUM)."""
                nc.vector.tensor_tensor(
                    ps[:, :, 0:7], ps[:, :, 0:7], gedge[:, si, 0], MUL)
                nc.vector.tensor_tensor(
                    ps[:, :, W - 7:W], ps[:, :, W - 7:W], gedge[:, si, 1], MUL)

            for _rep, img in [(r, i) for r in range(reps) for i in range(IMGS_PER_CORE)]:
                # ---- stage 1 prep: s = sum x_c, q = sum x_c^2 ----
                x = []
                for c in range(C):
                    xc = pers.tile([128, NB, W], F32, tag=f"x{c}")
                    nc.sync.dma_start(
                        xc[:], xs_ap[img, c].rearrange("(b p) w -> p b w", p=128))
                    x.append(xc)
                t0 = scr.tile([128, NB, W], F32, tag="scr")
                nc.vector.tensor_tensor(t0[:], x[0][:], x[1][:], ADD)
                nc.vector.tensor_tensor(ctr(s_t), t0[:], x[2][:], ADD)
                sq0 = scr.tile([128, NB, W], F32, tag="scr")
                nc.scalar.activation(sq0[:], x[0][:], SQUARE)
                sq1 = scr.tile([128, NB, W], F32, tag="scr")
                nc.scalar.activation(sq1[:], x[1][:], SQUARE)
                sq2 = scr.tile([128, NB, W], F32, tag="scr")
                nc.scalar.activation(sq2[:], x[2][:], SQUARE)
                t1 = scr.tile([128, NB, W], F32, tag="scr")
                nc.vector.tensor_tensor(t1[:], sq0[:], sq1[:], ADD)
                nc.vector.tensor_tensor(ctr(q_t), t1[:], sq2[:], ADD)

                G1 = H1 = G2 = Hh2 = u1 = u2 = u3 = v2 = v3 = None
                for si, (k, eps) in enumerate(STAGES):
                    # box(s), box(q) with normalization folded in
                    hs, pad = hscan(s_t, k)
                    S = vband(hs, pad, si, True, "S")   # ~mean after edgefix
                    edgefix(S, si)
                    hq, _ = hscan(q_t, k)
                    Q = vband(hq, pad, si, True, "Q")
                    edgefix(Q, si)

                    m2 = scr.tile([128, NB, W], F32, tag="scr")
                    nc.scalar.activation(m2[:], S[:], SQUARE)
                    den = scr.tile([128, NB, W], F32, tag="scr")
                    for b in range(NB):
                        nc.vector.affine_then_add(
                            den[:, b, :], m2[:, b, :], Q[:, b, :],
                            scale=-1.0, bias=eps)
                    r = scr.tile([128, NB, W], F32, tag="scr")
                    for b in range(NB):
                        nc.vector.reciprocal_approx_fast(r[:, b, :], den[:, b, :])
                    # A3 = 3 - 3*eps*r ; rm3 = (3*eps*r)*mean
                    nc.vector.tensor_scalar(
                        ctr(a_t), r[:], -3.0 * eps, 3.0, MUL, ADD)
                    nc.vector.scalar_tensor_tensor(
                        ctr(rm_t), r[:], 3.0 * eps, S[:], MUL, MUL)

                    ha, _ = hscan(a_t, k)
                    mA = vband(ha, pad, si, False, "mA")
                    hr, _ = hscan(rm_t, k)
                    mB = vband(hr, pad, si, False, "mB")

                    if si == 0:
                        G1 = pers.tile([128, NB, W], F32, tag="G1")
                        nc.scalar.activation(G1[:], mA[:], COPY)
                        H1 = pers.tile([128, NB, W], F32, tag="H1")
                        nc.scalar.activation(H1[:], mB[:], COPY)
                    elif si == 1:
                        G2 = pers.tile([128, NB, W], F32, tag="G2")
                        nc.vector.tensor_tensor(G2[:], mA[:], G1[:], MUL)
                        th = scr.tile([128, NB, W], F32, tag="scr")
                        nc.vector.tensor_tensor(th[:], mA[:], H1[:], MUL)
                        Hh2 = pers.tile([128, NB, W], F32, tag="H2")
                        nc.vector.tensor_tensor(Hh2[:], th[:], mB[:], ADD)
                        # u1, u2, v2 now (frees nothing yet but spreads work)
                        u1 = pers.tile([128, NB, W], F32, tag="u1")
                        nc.vector.tensor_scalar(u1[:], G1[:], -1.0, 1.0, MUL, ADD)
                        u2 = pers.tile([128, NB, W], F32, tag="u2")
                        nc.vector.tensor_tensor(u2[:], G1[:], G2[:], SUB)
                        v2 = pers.tile([128, NB, W], F32, tag="v2")
                        nc.vector.tensor_tensor(v2[:], H1[:], Hh2[:], SUB)
                    else:
                        w3 = scr.tile([128, NB, W], F32, tag="scr")
                        nc.vector.tensor_scalar(w3[:], mA[:], -1.0, 1.0, MUL, ADD)
                        u3 = pers.tile([128, NB, W], F32, tag="G1")  # G1 dead after u2
                        nc.vector.tensor_tensor(u3[:], w3[:], G2[:], MUL)
                        th3 = scr.tile([128, NB, W], F32, tag="scr")
                        nc.vector.tensor_tensor(th3[:], w3[:], Hh2[:], MUL)
                        v3 = pers.tile([128, NB, W], F32, tag="G2")  # G2 dead after u3
                        nc.vector.tensor_tensor(v3[:], th3[:], mB[:], SUB)

                    if si < 2:
                        # recurrence: s' = mA*s + 3mB ; q' = mA^2 q + 2 mA mB s + 3 mB^2
                        t1r = scr.tile([128, NB, W], F32, tag="scr")
                        nc.vector.tensor_tensor(t1r[:], mA[:], ctr(s_t), MUL)
                        am = scr.tile([128, NB, W], F32, tag="scr")
                        nc.scalar.activation(am[:], mA[:], SQUARE)
                        t2r = scr.tile([128, NB, W], F32, tag="scr")
                        nc.vector.tensor_tensor(t2r[:], am[:], ctr(q_t), MUL)
                        e = scr.tile([128, NB, W], F32, tag="scr")
                        nc.vector.tensor_tensor(e[:], mB[:], t1r[:], MUL)
                        q2 = scr.tile([128, NB, W], F32, tag="scr")
                        nc.vector.scalar_tensor_tensor(
                            q2[:], e[:], 2.0, t2r[:], MUL, ADD)
                        b2 = scr.tile([128, NB, W], F32, tag="scr")
                        nc.scalar.activation(b2[:], mB[:], SQUARE)
                        # order matters: write s' after t1r, q' after t2r
                        nc.vector.scalar_tensor_tensor(
                            ctr(s_t), mB[:], 3.0, t1r[:], MUL, ADD)
                        nc.vector.scalar_tensor_tensor(
                            ctr(q_t), b2[:], 3.0, q2[:], MUL, ADD)

                # ---- final: products + conv on PE (block-outer, o in banks) ----
                for b in range(NB):
                    cp = psum.tile([128, NB, W], F32, tag="ps")
                    for i in range(9):
                        j, c = divmod(i, 3)
                        uj = (u1, u2, u3)[j]
                        p = pp.tile([128, W], F32, tag="p")
                        nc.vector.tensor_tensor(
                            p[:], uj[:, b, :], x[c][:, b, :], MUL)
                        for o in range(3):
                            nc.tensor.matmul(
                                cp[:, o, :], diags[:, 9 * o + i, :], p[:],
                                start=(i == 0), stop=False,
                                skip_group_check=True,
                            )
                    for o in range(3):
                        vterms = [(27 + 3 * o + 0, v2[:, b, :]),
                                  (27 + 3 * o + 1, v3[:, b, :]),
                                  (27 + 3 * o + 2, H1[:, b, :])]
                        for t, (di, rap) in enumerate(vterms):
                            nc.tensor.matmul(
                                cp[:, o, :], diags[:, di, :], rap,
                                start=False, stop=(t == 2),
                                skip_group_check=True,
                            )
                        ob = obp.tile([128, W], F32, tag="ob")
                        nc.scalar.activation(ob[:], cp[:, o, :], COPY)
                        nc.sync.dma_start(
                            out_ap[img, o, 128 * b:128 * (b + 1), :], ob[:])

    nc.compile()
    return nc


_PROGRAM_CACHE = {}


def kernel(x_hr: np.ndarray, w1: np.ndarray) -> np.ndarray:
    from concourse import bass_utils

    assert x_hr.shape == (16, 3, 512, 512)
    nc = _PROGRAM_CACHE.get("nc")
    if nc is None:
        nc = _build_program()
        _PROGRAM_CACHE["nc"] = nc

    bands, gedges = _make_consts()
    diags = _make_diags(np.asarray(w1, np.float64))
    xs = np.ascontiguousarray(x_hr.reshape(N_CORES, IMGS_PER_CORE, C, H, W))
    in_maps = [
        {"xs": xs[i], "bands": bands, "gedge": gedges, "diags": diags}
        for i in range(N_CORES)
    ]
    res = bass_utils.run_bass_kernel_spmd(nc, in_maps, core_ids=list(range(N_CORES)))
    out = np.stack([res.results[i]["out"] for i in range(N_CORES)])
    return out.reshape(16, 3, 512, 512).astype(np.float32)



# revision 7
# speedup vs baseline: 261.0202x; 261.0202x over previous
"""DDFConvGuidedFilter Trainium2 kernel.

Data-parallel over batch: 16 images -> 8 cores x 2 images.

Per image (512x512, C=3), three guided-filter stages (k=3,7,15; eps=.16,.04,.01):
  s = sum_c x_c ; q = sum_c x_c^2
  per stage:  mean = box(s)/Nc ; Q = box(q)/Nc        (Nc = 3*cnt_h*cnt_w)
              var = Q - mean^2 ; r = 1/(var+eps)
              A3 = 3 - 3*eps*r (= 3A);  rm3 = 3*eps*r*mean (= 3b)
              mA = box(A3) ; mB = box(rm3)            (= C*box(A), C*box(b))
              next: s' = mA*s + 3mB ; q' = mA^2 q + 2 mA mB s + 3 mB^2
  F_j = G_j*x + H_j with G1=mA1, H1=mB1, G2=mA2*G1, H2=mA2*H1+mB2, ...
  out_o = sum_jc w1[o,3(j-1)+c] * (u_j * x_c)  + beta_o2*v2 + beta_o3*v3 - beta_o1*H1
    u1=1-G1, u2=G1-G2, u3=(1-mA3)*G2, v2=H1-H2, v3=(1-mA3)*H2-mB3

Box filters: horizontal pass = DVE tensor_tensor_scan (running sum of
x[t+pad]-x[t-pad-1], started `pad` early); vertical pass = PE matmul against
banded 128x128 Toeplitz blocks (with 1/Nc row-normalization folded into the
s/q bands; w-edge correction applied to 7 edge columns afterwards).
"""

import numpy as np

H = W = 512
C = 3
NB = 4  # h blocks of 128
OFF = 16  # left zero margin in padded tiles
BW = OFF + W + 8  # padded tile width = 536
STAGES = [(3, 0.16), (7, 0.04), (15, 0.01)]
N_CORES = 8
IMGS_PER_CORE = 2


def _tf32(a):
    """Round fp32 array to the fp32r (tf32, 11-bit mantissa) grid."""
    b = a.astype(np.float32).view(np.uint32)
    keep = np.uint32(0xFFFFE000)
    half = np.uint32(0x00001000)
    lsb = (b >> np.uint32(13)) & np.uint32(1)
    b = (b + half - np.uint32(1) + lsb) & keep
    return b.view(np.float32)


def _cnt(k, n=512):
    i = np.arange(n)
    p = (k - 1) // 2
    return np.minimum(i + p + 1, np.minimum(2 * p + 1, n - i + p)).astype(np.float64)


def _band_block(k, dj, row_scale=None):
    """lhsT[kk, m] = scale(h_out) * 1(|128*dj + kk - m| <= pad)."""
    pad = (k - 1) // 2
    m = np.arange(128)
    kk = np.arange(128)[:, None]
    mat = (np.abs(128 * dj + kk - m) <= pad).astype(np.float64)
    if row_scale is not None:
        mat = mat * row_scale[None, :]
    return mat


def _make_consts():
    """Band matrices [24,128,128] and edge-gamma tiles [3,2,128,4,7]."""
    bands = []
    gedges = np.zeros((3, 2, 128, NB, 7), np.float64)
    for si, (k, eps) in enumerate(STAGES):
        ch = _cnt(k)
        alpha = 1.0 / (3.0 * ch * k)  # row scale: 1/(3*cnt_h(h)*k)
        # sq bands: diag0, diagM, diag3, upM, dnM
        bands.append(_band_block(k, 0, alpha[0:128]))
        bands.append(_band_block(k, 0, alpha[128:256]))
        bands.append(_band_block(k, 0, alpha[384:512]))
        bands.append(_band_block(k, 1, alpha[128:256]))  # interior rows
        bands.append(_band_block(k, -1, alpha[128:256]))
        # A/rm bands: unnormalized
        bands.append(_band_block(k, 0))
        bands.append(_band_block(k, 1))
        bands.append(_band_block(k, -1))
        cw = _cnt(k)
        gl = k / cw[0:7]
        gr = k / cw[505:512]
        gedges[si, 0] = np.broadcast_to(gl[None, None, :], (128, NB, 7))
        gedges[si, 1] = np.broadcast_to(gr[None, None, :], (128, NB, 7))
    return _tf32(np.stack(bands).astype(np.float32)), gedges.astype(np.float32)


# band index helpers: per stage si, base = si*8
SQ_DIAG = {0: 0, 1: 1, 2: 1, 3: 2}  # per out-block
SQ_UP, SQ_DN = 3, 4
A_DIAG, A_UP, A_DN = 5, 6, 7


def _make_diags(w1):
    """Conv lhsT diag matrices [36,128,128]: 27 p-term + 9 v-term."""
    eye = np.eye(128, dtype=np.float64)
    mats = []
    for o in range(3):
        for j in range(3):
            for c in range(3):
                mats.append(w1[o, 3 * j + c] * eye)
    beta = w1.reshape(3, 3, 3).sum(axis=2)  # [o, j]
    for o in range(3):
        mats.append(beta[o, 1] * eye)   # v2
        mats.append(beta[o, 2] * eye)   # v3
        mats.append(-beta[o, 0] * eye)  # H1
    return _tf32(np.stack(mats).astype(np.float32))


def _build_program(reps=1):
    import concourse.bacc as bacc
    import concourse.tile as tile
    from concourse import mybir

    F32 = mybir.dt.float32
    F32R = mybir.dt.float32r
    ADD = mybir.AluOpType.add
    SUB = mybir.AluOpType.subtract
    MUL = mybir.AluOpType.mult
    SQUARE = mybir.ActivationFunctionType.Square
    COPY = mybir.ActivationFunctionType.Copy

    nc = bacc.Bacc("TRN2", target_bir_lowering=False, debug=False,
                   num_devices=N_CORES)
    xs_ap = nc.dram_tensor("xs", [IMGS_PER_CORE, C, H, W], F32,
                           kind="ExternalInput").ap()
    bands_ap = nc.dram_tensor("bands", [24, 128, 128], F32R,
                              kind="ExternalInput").ap()
    gedge_ap = nc.dram_tensor("gedge", [3, 2, 128, NB, 7], F32,
                              kind="ExternalInput").ap()
    diags_ap = nc.dram_tensor("diags", [36, 128, 128], F32R,
                              kind="ExternalInput").ap()
    out_ap = nc.dram_tensor("out", [IMGS_PER_CORE, C, H, W], F32,
                            kind="ExternalOutput").ap()

    with tile.TileContext(nc) as tc:
        with (
            tc.tile_pool(name="const", bufs=1) as constp,
            tc.tile_pool(name="fixed", bufs=1) as fixed,
            tc.tile_pool(name="scan", bufs=2) as scanp,
            tc.tile_pool(name="scr", bufs=4) as scr,
            tc.tile_pool(name="pers", bufs=1) as pers,
            tc.tile_pool(name="pp", bufs=4) as pp,
            tc.tile_pool(name="obp", bufs=3) as obp,
            tc.tile_pool(name="psum", bufs=2, space="PSUM") as psum,
        ):
            # ---- constants ----
            bands = constp.tile([128, 24, 128], F32R)
            nc.sync.dma_start(bands[:], bands_ap.rearrange("n k m -> k n m"))
            gedge = constp.tile([128, 3, 2, NB, 7], F32)
            nc.sync.dma_start(gedge[:], gedge_ap.rearrange("s e p b j -> p s e b j"))
            diags = constp.tile([128, 36, 128], F32R)
            nc.sync.dma_start(diags[:], diags_ap.rearrange("n k m -> k n m"))

            # ---- fixed padded map tiles (shared across stages & images) ----
            s_t = fixed.tile([128, NB, BW], F32, tag="s_t")
            q_t = fixed.tile([128, NB, BW], F32, tag="q_t")
            a_t = fixed.tile([128, NB, BW], F32, tag="a_t")
            rm_t = fixed.tile([128, NB, BW], F32, tag="rm_t")
            for t in (s_t, q_t, a_t, rm_t):
                nc.vector.memset(t[:], 0.0)

            def ctr(t):  # central (data) region of a padded tile
                return t[:, :, OFF:OFF + W]

            def hscan(padded, k):
                """H box pass -> scan tile [128, NB, 520]; data at [:, b, pad:pad+512]."""
                pad = (k - 1) // 2
                n = W + pad
                hs = scanp.tile([128, NB, 520], F32R, tag="hs")
                for b in range(NB):
                    nc.vector.tensor_tensor_scan(
                        hs[:, b, 0:n],
                        padded[:, b, OFF:OFF + n],
                        padded[:, b, OFF - k:OFF - k + n],
                        0.0, ADD, SUB,
                    )
                return hs, pad

            def vband(hs, pad, si, sq, tag):
                """V box pass on PE -> psum tile [128, NB, 512]."""
                ps = psum.tile([128, NB, W], F32, tag="ps")
                base = si * 8
                for b in range(NB):
                    js = [j for j in (b - 1, b, b + 1) if 0 <= j < NB]
                    for idx, j in enumerate(js):
                        if sq:
                            bi = base + (SQ_DIAG[b] if j == b else
                                         (SQ_UP if j == b + 1 else SQ_DN))
                        else:
                            bi = base + (A_DIAG if j == b else
                                         (A_UP if j == b + 1 else A_DN))
                        nc.tensor.matmul(
                            ps[:, b, :], bands[:, bi, :],
                            hs[:, j, pad:pad + W],
                            start=(idx == 0), stop=(idx == len(js) - 1),
                        )
                return ps

            def edgefix(ps, si):
                """multiply 7 left/right edge columns by gamma (in-place, PSUM)."""
                nc.vector.tensor_tensor(
                    ps[:, :, 0:7], ps[:, :, 0:7], gedge[:, si, 0], MUL)
                nc.vector.tensor_tensor(
                    ps[:, :, W - 7:W], ps[:, :, W - 7:W], gedge[:, si, 1], MUL)

            for _rep, img in [(r, i) for r in range(reps) for i in range(IMGS_PER_CORE)]:
                # ---- stage 1 prep: s = sum x_c, q = sum x_c^2 ----
                x = []
                for c in range(C):
                    xc = pers.tile([128, NB, W], F32, tag=f"x{c}")
                    nc.sync.dma_start(
                        xc[:], xs_ap[img, c].rearrange("(b p) w -> p b w", p=128))
                    x.append(xc)
                t0 = scr.tile([128, NB, W], F32, tag="scr")
                nc.vector.tensor_tensor(t0[:], x[0][:], x[1][:], ADD)
                nc.vector.tensor_tensor(ctr(s_t), t0[:], x[2][:], ADD)
                sq0 = scr.tile([128, NB, W], F32, tag="scr")
                nc.scalar.activation(sq0[:], x[0][:], SQUARE)
                sq1 = scr.tile([128, NB, W], F32, tag="scr")
                nc.scalar.activation(sq1[:], x[1][:], SQUARE)
                sq2 = scr.tile([128, NB, W], F32, tag="scr")
                nc.scalar.activation(sq2[:], x[2][:], SQUARE)
                t1 = scr.tile([128, NB, W], F32, tag="scr")
                nc.vector.tensor_tensor(t1[:], sq0[:], sq1[:], ADD)
                nc.vector.tensor_tensor(ctr(q_t), t1[:], sq2[:], ADD)

                G1 = H1 = G2 = Hh2 = u1 = u2 = u3 = v2 = v3 = None
                for si, (k, eps) in enumerate(STAGES):
                    # box(s), box(q) with normalization folded in
                    hs, pad = hscan(s_t, k)
                    S = vband(hs, pad, si, True, "S")   # ~mean after edgefix
                    edgefix(S, si)
                    hq, _ = hscan(q_t, k)
                    Q = vband(hq, pad, si, True, "Q")
                    edgefix(Q, si)

                    m2 = scr.tile([128, NB, W], F32, tag="scr")
                    nc.scalar.activation(m2[:], S[:], SQUARE)
                    den = scr.tile([128, NB, W], F32, tag="scr")
                    for b in range(NB):
                        nc.vector.affine_then_add(
                            den[:, b, :], m2[:, b, :], Q[:, b, :],
                            scale=-1.0, bias=eps)
                    r = scr.tile([128, NB, W], F32, tag="scr")
                    for b in range(NB):
                        nc.vector.reciprocal_approx_fast(r[:, b, :], den[:, b, :])
                    # A3 = 3 - 3*eps*r ; rm3 = (3*eps*r)*mean
                    nc.vector.tensor_scalar(
                        ctr(a_t), r[:], -3.0 * eps, 3.0, MUL, ADD)
                    nc.vector.scalar_tensor_tensor(
                        ctr(rm_t), r[:], 3.0 * eps, S[:], MUL, MUL)

                    ha, _ = hscan(a_t, k)
                    mA = vband(ha, pad, si, False, "mA")
                    hr, _ = hscan(rm_t, k)
                    mB = vband(hr, pad, si, False, "mB")

                    if si == 0:
                        G1 = pers.tile([128, NB, W], F32, tag="G1")
                        nc.scalar.activation(G1[:], mA[:], COPY)
                        H1 = pers.tile([128, NB, W], F32R, tag="H1")
                        nc.scalar.activation(H1[:], mB[:], COPY)
                    elif si == 1:
                        G2 = pers.tile([128, NB, W], F32, tag="G2")
                        nc.vector.tensor_tensor(G2[:], mA[:], G1[:], MUL)
                        th = scr.tile([128, NB, W], F32, tag="scr")
                        nc.vector.tensor_tensor(th[:], mA[:], H1[:], MUL)
                        Hh2 = pers.tile([128, NB, W], F32, tag="H2")
                        nc.vector.tensor_tensor(Hh2[:], th[:], mB[:], ADD)
                        # u1, u2, v2 now (frees nothing yet but spreads work)
                        u1 = pers.tile([128, NB, W], F32, tag="u1")
                        nc.vector.tensor_scalar(u1[:], G1[:], -1.0, 1.0, MUL, ADD)
                        u2 = pers.tile([128, NB, W], F32, tag="u2")
                        nc.vector.tensor_tensor(u2[:], G1[:], G2[:], SUB)
                        v2 = pers.tile([128, NB, W], F32R, tag="v2")
                        nc.vector.tensor_tensor(v2[:], H1[:], Hh2[:], SUB)
                    else:
                        w3 = scr.tile([128, NB, W], F32, tag="scr")
                        nc.vector.tensor_scalar(w3[:], mA[:], -1.0, 1.0, MUL, ADD)
                        u3 = pers.tile([128, NB, W], F32, tag="G1")  # G1 dead after u2
                        nc.vector.tensor_tensor(u3[:], w3[:], G2[:], MUL)
                        th3 = scr.tile([128, NB, W], F32, tag="scr")
                        nc.vector.tensor_tensor(th3[:], w3[:], Hh2[:], MUL)
                        v3 = pers.tile([128, NB, W], F32R, tag="G2")  # G2 dead after u3
                        nc.vector.tensor_tensor(v3[:], th3[:], mB[:], SUB)

                    if si < 2:
                        # recurrence: s' = mA*s + 3mB ; q' = mA^2 q + 2 mA mB s + 3 mB^2
                        t1r = scr.tile([128, NB, W], F32, tag="scr")
                        nc.vector.tensor_tensor(t1r[:], mA[:], ctr(s_t), MUL)
                        am = scr.tile([128, NB, W], F32, tag="scr")
                        nc.scalar.activation(am[:], mA[:], SQUARE)
                        t2r = scr.tile([128, NB, W], F32, tag="scr")
                        nc.vector.tensor_tensor(t2r[:], am[:], ctr(q_t), MUL)
                        e = scr.tile([128, NB, W], F32, tag="scr")
                        nc.vector.tensor_tensor(e[:], mB[:], t1r[:], MUL)
                        q2 = scr.tile([128, NB, W], F32, tag="scr")
                        nc.vector.scalar_tensor_tensor(
                            q2[:], e[:], 2.0, t2r[:], MUL, ADD)
                        b2 = scr.tile([128, NB, W], F32, tag="scr")
                        nc.scalar.activation(b2[:], mB[:], SQUARE)
                        # order matters: write s' after t1r, q' after t2r
                        nc.vector.scalar_tensor_tensor(
                            ctr(s_t), mB[:], 3.0, t1r[:], MUL, ADD)
                        nc.vector.scalar_tensor_tensor(
                            ctr(q_t), b2[:], 3.0, q2[:], MUL, ADD)

                # ---- final: products + conv on PE (block-outer, o in banks) ----
                for b in range(NB):
                    cp = psum.tile([128, NB, W], F32, tag="ps")
                    for i in range(9):
                        j, c = divmod(i, 3)
                        uj = (u1, u2, u3)[j]
                        p = pp.tile([128, W], F32R, tag="p")
                        nc.vector.tensor_tensor(
                            p[:], uj[:, b, :], x[c][:, b, :], MUL)
                        for o in range(3):
                            nc.tensor.matmul(
                                cp[:, o, :], diags[:, 9 * o + i, :],
                                p[:],
                                start=(i == 0), stop=False,
                                skip_group_check=True,
                            )
                    for o in range(3):
                        vterms = [(27 + 3 * o + 0, v2[:, b, :]),
                                  (27 + 3 * o + 1, v3[:, b, :]),
                                  (27 + 3 * o + 2, H1[:, b, :])]
                        for t, (di, rap) in enumerate(vterms):
                            nc.tensor.matmul(
                                cp[:, o, :], diags[:, di, :],
                                rap,
                                start=False, stop=(t == 2),
                                skip_group_check=True,
                            )
                        ob = obp.tile([128, W], F32, tag="ob")
                        nc.scalar.activation(ob[:], cp[:, o, :], COPY)
                        nc.sync.dma_start(
                            out_ap[img, o, 128 * b:128 * (b + 1), :], ob[:])

    nc.compile()
    return nc


_PROGRAM_CACHE = {}


def kernel(x_hr: np.ndarray, w1: np.ndarray) -> np.ndarray:
    from concourse import bass_utils

    assert x_hr.shape == (16, 3, 512, 512)
    nc = _PROGRAM_CACHE.get("nc")
    if nc is None:
        nc = _build_program()
        _PROGRAM_CACHE["nc"] = nc

    bands, gedges = _make_consts()
    diags = _make_diags(np.asarray(w1, np.float64))
    xs = np.ascontiguousarray(x_hr.reshape(N_CORES, IMGS_PER_CORE, C, H, W))
    in_maps = [
        {"xs": xs[i], "bands": bands, "gedge": gedges, "diags": diags}
        for i in range(N_CORES)
    ]
    res = bass_utils.run_bass_kernel_spmd(nc, in_maps, core_ids=list(range(N_CORES)))
    out = np.stack([res.results[i]["out"] for i in range(N_CORES)])
    return out.reshape(16, 3, 512, 512).astype(np.float32)

